# revision 63
# baseline (speedup 1.0000x reference)
"""Linear attention (silu+1 feature map) MultiHeadAttention kernel for 8x TRN2.

Sharding: data-parallel over batch (B=8 -> 1 batch element per NeuronCore).

fp8 DoubleRow design. All four big GEMMs run as fp8e4 DoubleRow matmuls
(contraction 256 per instruction at 0.5 cycles/output-row). Accuracy is
preserved by a mean-split: phi(x) = silu(x)+1 = 1 + d(x), so every fp8
operand that multiplies a full-magnitude tensor is a small delta:

  phase 1 (token tiles of 512, q-tiles and kv-subs interleaved per pair):
    qT[o,t]  = wq8.T @ x8T            (DoubleRow, feature-major out)
    dq[o,t]  = silu(s*qT + s*bq)      (ACT, fp8 out; phi_q = 1 + dq)
    k[t,o]   = x8T.T @ wk8            (DoubleRow, token-major)
    v[t,e]   = x8T.T @ (Wv/4)8 + bv/4 (DoubleRow + DVE bias add, fp8 out;
                                       the /4 keeps kv inside fp8e4 range)
    dk[t,o]  = silu(s*k)              (ACT, fp8 out; phi_k = 1 + dk)
    kv[e,d]_h += v_h.T @ dk_h         (DoubleRow over token sub-tile pairs;
                                       [64,16,64] psum, head h in slot
                                       (h%2)*8 + h//2 so even/odd heads are
                                       contiguous for the 2-op repack)
  corrections (the "1" parts of phi_k):
    kv_full  = kv + S_v[e]/4          S_v = sum_t v_exact[t] computed on host
                                      from the original fp32 x (rank-1 term)
  M stage:
    kv8      = block-diag fp8 repack of kv_full  ([64, 2, 128] per chunk)
    M[d,o]_c = kv8_c .T@ (4*wo2)_pair (DoubleRow, contraction = 2 heads' e)
    m8       = fp8(M)                 (exact scale: (kv/4) @ (4 wo))
  colsum (the "1" part of phi_q): y += colsum(M) as a per-partition bias:
    rsum[e]  = reduce_d kv_ps + 16*S_v  (DVE reduce + add, = rowsum(kv)/4)
    bias[o]  = sum_e rsum[e] (4*wo)[e,o] + bo  (128 N=1 matvecs, bf16)
  phase 2:
    yT[o,t]  = m8.T @ dq + bias       (DoubleRow; bias folded into drains)

Host side: fp8 casts (clip +-240), S_v correction, output upcast bf16->fp32.
"""

import numpy as np
import ml_dtypes

B, T, D = 8, 4096, 1024
H, DH = 16, 64
SCALE = float(DH ** -0.25)
NCORES = 8
P = 128
DC = D // P          # 8 feature chunks
NKP = DC // 2        # 4 k-pairs for DoubleRow
TT = 512             # token tile (phase 1)
NTT = T // TT        # 8 token tiles
NSUB = TT // P       # 4 sub-tiles of 128 tokens

_BF16 = ml_dtypes.bfloat16
_F8 = ml_dtypes.float8_e4m3fn

_CACHE = {}


def _slot(h):
    # kv psum column slot for head h: even heads 0-7, odd heads 8-15
    return (h % 2) * 8 + h // 2


def _split_multi_waits(nc):
    """walrus in this container only encodes ONE sync-wait command per
    instruction. Hoist extra waits onto injected same-engine NOPs placed
    immediately before the instruction (program order on the engine queue
    makes this semantically identical)."""
    import concourse.mybir as mybir

    n_split = 0
    for fn in nc.m.functions:
        for bb in fn.blocks:
            new = []
            changed = False
            for inst in bb.instructions:
                si = inst.sync_info
                waits = list(si.on_wait) if si is not None else []
                if len(waits) > 1:
                    changed = True
                    for j, w in enumerate(waits[:-1]):
                        nop = mybir.InstNoOp(
                            name=f"{inst.name}-sw{j}", ins=[], outs=[]
                        )
                        nop.engine = inst.engine
                        nop.sync_info = mybir.SyncInfo(
                            on_wait=[w], on_update=[]
                        )
                        new.append(nop)
                        n_split += 1
                    inst.sync_info = mybir.SyncInfo(
                        on_wait=[waits[-1]], on_update=list(si.on_update)
                    )
                new.append(inst)
            if changed:
                bb.instructions = new
    return n_split


def _build_program():
    import concourse.bass as bass
    import concourse.mybir as mybir
    from concourse.tile import TileContext

    dt = mybir.dt
    AF = mybir.ActivationFunctionType
    PM = mybir.MatmulPerfMode

    nc = bass.Bass()

    xT_d = nc.dram_tensor("xT", [D, T], dt.float8e4, kind="ExternalInput")
    wq_d = nc.dram_tensor("wq", [D, D], dt.float8e4, kind="ExternalInput")
    wk_d = nc.dram_tensor("wk", [D, D], dt.float8e4, kind="ExternalInput")
    wv_d = nc.dram_tensor("wv", [D, D], dt.float8e4, kind="ExternalInput")
    wo2_d = nc.dram_tensor("wo2", [64, H, D], dt.float8e4, kind="ExternalInput")
    wob2_d = nc.dram_tensor("wob2", [64, H, D], dt.bfloat16, kind="ExternalInput")
    bqs_d = nc.dram_tensor("bqs", [P, DC], dt.float32, kind="ExternalInput")
    bos_d = nc.dram_tensor("bos", [P, DC], dt.float32, kind="ExternalInput")
    bvb_d = nc.dram_tensor("bvb", [P, D], dt.float32, kind="ExternalInput")
    corA_d = nc.dram_tensor("corA", [64, 8, 64], dt.float32, kind="ExternalInput")
    corB_d = nc.dram_tensor("corB", [64, 8, 64], dt.float32, kind="ExternalInput")
    cor64_d = nc.dram_tensor("cor64", [64, H], dt.float32, kind="ExternalInput")
    yT_d = nc.dram_tensor("yT", [D, T], dt.bfloat16, kind="ExternalOutput")

    with TileContext(nc) as tc:
        with (
            tc.tile_pool(name="weights", bufs=1) as wpool,
            tc.tile_pool(name="phiq", bufs=1) as qpool,
            tc.tile_pool(name="msb", bufs=1) as mpool,
            tc.tile_pool(name="xin", bufs=3) as xpool,
            tc.tile_pool(name="kvtiles", bufs=4) as kvpool,
            tc.tile_pool(name="yout", bufs=4) as ypool,
        ):
            # ---- weight / const preload ----
            wq_sb = wpool.tile([P, DC, D], dt.float8e4, tag="wq")
            wk_sb = wpool.tile([P, DC, D], dt.float8e4, tag="wk")
            wv_sb = wpool.tile([P, DC, D], dt.float8e4, tag="wv")
            wo2_sb = wpool.tile([64, H, D], dt.float8e4, tag="wo2")
            wob2_sb = wpool.tile([64, H, D], dt.bfloat16, tag="wob2")
            bq_sb = wpool.tile([P, DC], dt.float32, tag="bq")
            bo_sb = wpool.tile([P, DC], dt.float32, tag="bo")
            bv_sb = wpool.tile([P, D], dt.float32, tag="bv")
            corA_sb = wpool.tile([64, 8, 64], dt.float32, tag="corA")
            corB_sb = wpool.tile([64, 8, 64], dt.float32, tag="corB")
            cor64_sb = wpool.tile([64, H], dt.float32, tag="cor64")
            bias_sb = wpool.tile([P, DC], dt.float32, tag="bias")
            rsum_sb = wpool.tile([64, H], dt.bfloat16, tag="rsum")
            rsraw_sb = wpool.tile([64, H], dt.float32, tag="rsraw")
            qraw_sb = wpool.tile([P, DC, TT], dt.bfloat16, tag="qraw")

            wq_r = wq_d.rearrange("(c p) o -> p c o", p=P)
            xT_r = xT_d.rearrange("(c p) t -> p c t", p=P)

            # sync queue: what phase-1 pair 0's q tiles need, in need-order.
            # gpsimd queue: wk/wv (needed ~11us in) + consts + x pairs 1-3.
            xt01 = xpool.tile([P, DC, 2 * TT], dt.float8e4, tag="xt0", bufs=1)
            nc.sync.dma_start(xt01[:, :, 0:TT], xT_r[:, :, 0:TT])
            nc.sync.dma_start(wq_sb[:, :, 0:512], wq_r[:, :, 0:512])
            nc.sync.dma_start(xt01[:, :, TT : 2 * TT], xT_r[:, :, TT : 2 * TT])
            nc.sync.dma_start(wq_sb[:, :, 512:1024], wq_r[:, :, 512:1024])
            nc.sync.dma_start(bv_sb[:], bvb_d[:])
            nc.sync.dma_start(bo_sb[:], bos_d[:])
            nc.sync.dma_start(wo2_sb[:], wo2_d[:])
            nc.sync.dma_start(wob2_sb[:], wob2_d[:])
            nc.gpsimd.dma_start(bq_sb[:], bqs_d[:])
            nc.gpsimd.dma_start(wk_sb[:], wk_d.rearrange("(c p) o -> p c o", p=P))
            nc.gpsimd.dma_start(wv_sb[:], wv_d.rearrange("(c p) o -> p c o", p=P))
            nc.gpsimd.dma_start(corA_sb[:], corA_d[:])
            nc.gpsimd.dma_start(corB_sb[:], corB_d[:])
            nc.gpsimd.dma_start(cor64_sb[:], cor64_d[:])
            xt_pre = [xt01[:, :, 0:TT], xt01[:, :, TT : 2 * TT]]

            phi_q = qpool.tile([P, DC, T], dt.float8e4, tag="phiq")
            m8 = mpool.tile([P, DC, D], dt.float8e4, tag="m8")
            # block-diag fp8 repack of kv: chunk c = [64, 2, 128]; j=0 holds
            # head 2c in cols 0:64, j=1 holds head 2c+1 in cols 64:128, rest 0
            kv8 = mpool.tile([64, DC, 2, P], dt.float8e4, tag="kv8")

            zz = wpool.tile([1, 640], dt.bfloat16, tag="zz")
            # zdum only feeds warmup matmuls (whose outputs are later
            # overwritten by start=True matmuls); a fast gpsimd memset
            # unblocks the PE ~1us earlier than the DVE zz memset
            zdum = wpool.tile([1, 640], dt.bfloat16, tag="zdum")
            nc.gpsimd.memset(zdum[:], 0.0)
            nc.vector.memset(zz[:], 0.0)
            nc.gpsimd.memset(kv8[:], 0.0)

            with tc.tile_pool(name="ps_kv", bufs=1, space="PSUM") as pkv_pool:
                # kv accumulator: head h in column slot (h%2)*8 + h//2
                kv_ps = pkv_pool.tile([64, H, 64], dt.float32, tag="kvacc")

                # warmup matmuls filling the startup DMA shadow: keep the PE
                # p-state warm so the first real matmuls run at full clock.
                # They scribble garbage into the kv bank; the first real kv
                # accumulation per head slot uses start=True to overwrite.
                for w in range(44):
                    nc.tensor.matmul(
                        kv_ps[:, 0:2, :], lhsT=zdum[:1, :64], rhs=zdum[:1, 64:192],
                        start=True, stop=True, skip_group_check=True,
                    )

                kv_pend = [None]
                kv_first = [True]

                def _emit_kv(pending, last):
                    v2_p, dk2_p = pending
                    for h in range(H):
                        nc.tensor.matmul(
                            kv_ps[:, _slot(h), :],
                            lhsT=v2_p[:, :, h * 64 : (h + 1) * 64],
                            rhs=dk2_p[:, :, h * 64 : (h + 1) * 64],
                            start=kv_first[0],
                            stop=last and h == H - 1,
                            skip_group_check=True,
                            perf_mode=PM.DoubleRow,
                        )
                    kv_first[0] = False

                with tc.tile_pool(name="ps_big", bufs=3, space="PSUM") as pbig:

                    def _q_tile(pair, xts, oc, defer_half=False):
                        pq = pbig.tile([P, 2 * TT], dt.float32, tag="pbig")
                        for half in range(2):
                            for kp in range(NKP):
                                nc.tensor.matmul(
                                    pq[:, half * TT : (half + 1) * TT],
                                    lhsT=wq_sb[:, 2 * kp : 2 * kp + 2, oc * P : (oc + 1) * P],
                                    rhs=xts[half][:, 2 * kp : 2 * kp + 2, :],
                                    start=(kp == 0),
                                    stop=(kp == NKP - 1),
                                    perf_mode=PM.DoubleRow,
                                )
                        if defer_half:
                            # ACT is saturated in pair 0's q stretch: silu
                            # the first token half now, park the second as
                            # raw bf16 via the idle DVE and silu it during
                            # the kv section
                            nc.scalar.activation(
                                phi_q[:, oc, pair * 2 * TT : pair * 2 * TT + TT],
                                pq[:, 0:TT], AF.Silu,
                                bias=bq_sb[:, oc : oc + 1], scale=SCALE,
                            )
                            nc.vector.tensor_copy(
                                out=qraw_sb[:, oc, :], in_=pq[:, TT : 2 * TT]
                            )
                        else:
                            nc.scalar.activation(
                                phi_q[:, oc, pair * 2 * TT : (pair + 1) * 2 * TT],
                                pq[:], AF.Silu,
                                bias=bq_sb[:, oc : oc + 1], scale=SCALE,
                            )

                    def _q_deferred(pair, oc):
                        nc.scalar.activation(
                            phi_q[:, oc, pair * 2 * TT + TT : (pair + 1) * 2 * TT],
                            qraw_sb[:, oc, :], AF.Silu,
                            bias=bq_sb[:, oc : oc + 1], scale=SCALE,
                        )

                    kv_cur = [None, None]

                    def _kv_sub(pair, xts, g):
                        half, sub = divmod(g, NSUB)
                        j = g % 2
                        xs = xts[half][:, :, sub * P : (sub + 1) * P]
                        pk = pbig.tile([P, D], dt.float32, tag="pbig")
                        pv = pbig.tile([P, D], dt.float32, tag="pbig")
                        for n in range(2):
                            for kp in range(NKP):
                                nc.tensor.matmul(
                                    pk[:, n * 512 : (n + 1) * 512],
                                    lhsT=xs[:, 2 * kp : 2 * kp + 2, :],
                                    rhs=wk_sb[:, 2 * kp : 2 * kp + 2, n * 512 : (n + 1) * 512],
                                    start=(kp == 0),
                                    stop=(kp == NKP - 1),
                                    perf_mode=PM.DoubleRow,
                                )
                        for n in range(2):
                            for kp in range(NKP):
                                nc.tensor.matmul(
                                    pv[:, n * 512 : (n + 1) * 512],
                                    lhsT=xs[:, 2 * kp : 2 * kp + 2, :],
                                    rhs=wv_sb[:, 2 * kp : 2 * kp + 2, n * 512 : (n + 1) * 512],
                                    start=(kp == 0),
                                    stop=(kp == NKP - 1),
                                    perf_mode=PM.DoubleRow,
                                )
                        if j == 0:
                            kv_cur[0] = kvpool.tile(
                                [P, 2, D], dt.float8e4, tag="v2", name="v2"
                            )
                            kv_cur[1] = kvpool.tile(
                                [P, 2, D], dt.float8e4, tag="dk2", name="dk2"
                            )
                        v2, dk2 = kv_cur
                        nc.scalar.activation(
                            dk2[:, j, :], pk[:], AF.Silu, scale=SCALE
                        )
                        nc.vector.tensor_add(v2[:, j, :], pv[:], bv_sb[:])
                        if j == 1:
                            if kv_pend[0] is not None:
                                _emit_kv(kv_pend[0], False)
                            kv_pend[0] = (v2, dk2)

                    def _hook_flush():
                        _emit_kv(kv_pend[0], True)
                        kv_pend[0] = None

                    def _hook_repack():
                        # kv_ps holds kv/4 (v pre-scaled); straight add+cast.
                        # gpsimd cannot read PSUM, so everything is ACT/DVE;
                        # chunks 0-1 go first since the M stage gates on them
                        nc.vector.tensor_add(
                            kv8[:, 0:2, 0, 0:64], kv_ps[:, 0:2, :],
                            corA_sb[:, 0:2, :],
                        )
                        nc.vector.tensor_add(
                            kv8[:, 0:2, 1, 64:128], kv_ps[:, 8:10, :],
                            corB_sb[:, 0:2, :],
                        )
                        nc.vector.tensor_add(
                            kv8[:, 2:8, 0, 0:64], kv_ps[:, 2:8, :],
                            corA_sb[:, 2:8, :],
                        )
                        nc.vector.tensor_add(
                            kv8[:, 2:8, 1, 64:128], kv_ps[:, 10:16, :],
                            corB_sb[:, 2:8, :],
                        )
                        nc.vector.tensor_reduce(
                            rsraw_sb[:], kv_ps[:],
                            axis=mybir.AxisListType.X,
                            op=mybir.AluOpType.add,
                        )
                        nc.vector.tensor_add(
                            rsum_sb[:], rsraw_sb[:], cor64_sb[:]
                        )

                    def _m_stage(cs):
                        # half-drains on ACT+DVE in parallel free the psum
                        # slot ~2x sooner than a single 1024-wide copy
                        for c in cs:
                            pm = pbig.tile([P, D], dt.float32, tag="pbig")
                            for n in range(2):
                                nc.tensor.matmul(
                                    pm[:, n * 512 : (n + 1) * 512],
                                    lhsT=kv8[:, c, :, :],
                                    rhs=wo2_sb[:, 2 * c : 2 * c + 2, n * 512 : (n + 1) * 512],
                                    start=True,
                                    stop=True,
                                    perf_mode=PM.DoubleRow,
                                )
                            nc.scalar.copy(out=m8[:, c, 0:512], in_=pm[:, 0:512])
                            nc.vector.tensor_copy(
                                out=m8[:, c, 512:1024], in_=pm[:, 512:1024]
                            )

                    def _hook_bias():
                        bt = pbig.tile([P, D], dt.float32, tag="pbig")
                        nc.tensor.matmul(
                            bt[:, 0:8], lhsT=zz[:1, :P], rhs=zz[:1, P : P + 8],
                            start=True, stop=True, skip_group_check=True,
                        )
                        for oc in range(DC):
                            for jj in range(H):
                                nc.tensor.matmul(
                                    bt[:, oc : oc + 1],
                                    lhsT=wob2_sb[:, jj, oc * P : (oc + 1) * P],
                                    rhs=rsum_sb[:, jj : jj + 1],
                                    start=False,
                                    stop=(oc == DC - 1 and jj == H - 1),
                                    skip_group_check=True,
                                )
                        nc.vector.tensor_add(bias_sb[:], bt[:, 0:8], bo_sb[:])

                    # ---- phase-1 pair schedules ----
                    for pair in range(NTT // 2):
                        if pair == 0:
                            xts = xt_pre
                        else:
                            xts = []
                            for half in range(2):
                                tt = pair * 2 + half
                                xt = xpool.tile([P, DC, TT], dt.float8e4, tag="xt")
                                nc.gpsimd.dma_start(
                                    xt[:], xT_r[:, :, tt * TT : (tt + 1) * TT]
                                )
                                xts.append(xt)

                        if pair == 0:
                            # startup: q tiles first (kv weights still
                            # loading), then the kv subs with the deferred
                            # second-half silus woven in
                            for oc in range(DC):
                                _q_tile(pair, xts, oc, defer_half=True)
                            for g in range(2 * NSUB):
                                _kv_sub(pair, xts, g)
                                _q_deferred(pair, g)
                        elif pair < NTT // 2 - 1:
                            # interleave 1:1 to balance the ACT engine
                            for u in range(DC):
                                _q_tile(pair, xts, u)
                                _kv_sub(pair, xts, u)
                        else:
                            # last pair: all kv subs first so the kv flush,
                            # repack, M stage and colsum hide under the q
                            # tiles that run right before phase 2.
                            for g in range(2 * NSUB):
                                _kv_sub(pair, xts, g)
                            _q_tile(pair, xts, 0)
                            _hook_flush()
                            _q_tile(pair, xts, 1)
                            _hook_repack()
                            _q_tile(pair, xts, 2)
                            _m_stage((0, 1))
                            _q_tile(pair, xts, 3)
                            _m_stage((2, 3))
                            _q_tile(pair, xts, 4)
                            _m_stage((4, 5))
                            _q_tile(pair, xts, 5)
                            _m_stage((6, 7))
                            _hook_bias()
                            _q_tile(pair, xts, 6)
                            _q_tile(pair, xts, 7)

                    # ---- phase 2: yT = m8.T @ dq + bias ----
                    # kp rotated per tile so the first tiles only need the
                    # early m8 chunks. Two qb tiles share one [P, 2048]
                    # output buffer -> half the DMAs at double the size;
                    # queues alternate sync/gpsimd. The last tile drains in
                    # 512-col pieces with its own small DMAs so the kernel
                    # tail stays short.
                    nt2 = 0
                    for oc in range(DC):
                        for qp in range(2):
                            # last two buffer-pairs use small per-half
                            # transfers so the DMA pipe drains quickly at
                            # the kernel end
                            idx = oc * 2 + qp
                            last_pair = idx >= 14
                            ysb = ypool.tile([P, 4 * TT], dt.bfloat16, tag="ysb")
                            for qh in range(2):
                                qb = qp * 2 + qh
                                py = pbig.tile([P, 2 * TT], dt.float32, tag="pbig")
                                for half in range(2):
                                    for i in range(NKP):
                                        kp = (nt2 + i) % NKP
                                        nc.tensor.matmul(
                                            py[:, half * TT : (half + 1) * TT],
                                            lhsT=m8[:, 2 * kp : 2 * kp + 2, oc * P : (oc + 1) * P],
                                            rhs=phi_q[:, 2 * kp : 2 * kp + 2,
                                                      qb * 1024 + half * TT : qb * 1024 + (half + 1) * TT],
                                            start=(i == 0),
                                            stop=(i == NKP - 1),
                                            perf_mode=PM.DoubleRow,
                                        )
                                nt2 += 1
                                # full-tile drains alternating ACT/DVE keep
                                # both engines under ~75% so neither builds
                                # a backlog; the final tile splits for a
                                # short tail.
                                o0 = qh * 2 * TT
                                if last_pair:
                                    # final pairs: fine-grained half drains,
                                    # transfers fanned over the queues
                                    nc.scalar.activation(
                                        ysb[:, o0 : o0 + TT], py[:, 0:TT],
                                        AF.Identity,
                                        bias=bias_sb[:, oc : oc + 1], scale=1.0,
                                    )
                                    nc.vector.tensor_scalar_add(
                                        ysb[:, o0 + TT : o0 + 2 * TT],
                                        py[:, TT : 2 * TT],
                                        bias_sb[:, oc : oc + 1],
                                    )
                                    q0 = nc.sync if qh == 0 else nc.scalar
                                    q1 = nc.gpsimd if qh == 0 else nc.sync
                                    q0.dma_start(
                                        yT_d[oc * P : (oc + 1) * P,
                                             qb * 1024 : qb * 1024 + TT],
                                        ysb[:, o0 : o0 + TT],
                                    )
                                    q1.dma_start(
                                        yT_d[oc * P : (oc + 1) * P,
                                             qb * 1024 + TT : (qb + 1) * 1024],
                                        ysb[:, o0 + TT : o0 + 2 * TT],
                                    )
                                elif qh == 0:
                                    nc.scalar.activation(
                                        ysb[:, 0 : 2 * TT], py[:],
                                        AF.Identity,
                                        bias=bias_sb[:, oc : oc + 1], scale=1.0,
                                    )
                                else:
                                    nc.vector.tensor_scalar_add(
                                        ysb[:, 2 * TT : 4 * TT], py[:],
                                        bias_sb[:, oc : oc + 1],
                                    )
                            if last_pair:
                                pass
                            elif (oc * 2 + qp) % 2 == 0:
                                nc.sync.dma_start(
                                    yT_d[oc * P : (oc + 1) * P,
                                         qp * 2048 : (qp + 1) * 2048],
                                    ysb[:],
                                )
                            else:
                                nc.gpsimd.dma_start(
                                    yT_d[oc * P : (oc + 1) * P,
                                         qp * 2048 : (qp + 1) * 2048],
                                    ysb[:],
                                )
    _split_multi_waits(nc)
    return nc


def _get_program():
    key = "nc"
    if key not in _CACHE:
        _CACHE[key] = _build_program()
    return _CACHE[key]


def _q8(a):
    return np.clip(np.asarray(a, np.float32), -240.0, 240.0).astype(_F8)


def _head_order():
    # slot j -> head: even heads first (0,2,..,14), then odd (1,3,..,15)
    return [2 * j for j in range(8)] + [2 * j + 1 for j in range(8)]


def _prep_shared(Wq, bq, Wk, Wv, bv, Wo, bo):
    woT = np.ascontiguousarray(Wo.T)  # [e, o]
    wo_heads = woT.reshape(H, 64, D)
    shared = {
        "wq": _q8(np.ascontiguousarray(Wq.T)),
        "wk": _q8(np.ascontiguousarray(Wk.T)),
        "wv": _q8(0.25 * np.ascontiguousarray(Wv.T)),
        "wo2": _q8(4.0 * wo_heads.transpose(1, 0, 2)),
        "wob2": np.ascontiguousarray(
            4.0 * wo_heads[_head_order()].transpose(1, 0, 2)
        ).astype(_BF16),
        "bqs": np.ascontiguousarray(
            (SCALE * bq).astype(np.float32).reshape(DC, P).T
        ),
        "bos": np.ascontiguousarray(bo.astype(np.float32).reshape(DC, P).T),
        "bvb": np.ascontiguousarray(
            np.broadcast_to(0.25 * bv.astype(np.float32), (P, D))
        ),
    }
    return shared


def _run(in_maps, **kw):
    from concourse.bass_utils import run_bass_kernel_spmd

    nc = _get_program()
    return run_bass_kernel_spmd(nc, in_maps, list(range(NCORES)), **kw)


def kernel(x, Wq, bq, Wk, Wv, bv, Wo, bo):
    x = np.asarray(x, dtype=np.float32)
    assert x.shape == (B, T, D), x.shape
    Wv = np.asarray(Wv, np.float32)
    bv = np.asarray(bv, np.float32)
    shared = _prep_shared(
        np.asarray(Wq, np.float32), np.asarray(bq, np.float32),
        np.asarray(Wk, np.float32), Wv, bv,
        np.asarray(Wo, np.float32), np.asarray(bo, np.float32),
    )
    ho = _head_order()
    in_maps = []
    for b in range(B):
        m = dict(shared)
        m["xT"] = _q8(x[b].T)
        # exact rank-1 kv correction: S_v[e] = sum_t v_exact[t, e]; kv is
        # accumulated at 1/4 scale on-device
        sv = (x[b].astype(np.float64).sum(0) @ Wv.T.astype(np.float64)
              + T * bv.astype(np.float64)).astype(np.float32)
        svh = 0.25 * sv.reshape(H, 64)                    # [head, e] / 4
        m["corA"] = np.ascontiguousarray(
            np.broadcast_to(svh[0::2].transpose(1, 0)[:, :, None], (64, 8, 64))
        )
        m["corB"] = np.ascontiguousarray(
            np.broadcast_to(svh[1::2].transpose(1, 0)[:, :, None], (64, 8, 64))
        )
        m["cor64"] = np.ascontiguousarray(64.0 * svh[ho].transpose(1, 0))
        in_maps.append(m)

    res = _run(in_maps)
    out = np.empty((B, T, D), np.float32)
    for b in range(B):
        out[b] = res.results[b]["yT"].astype(np.float32).T
    return out


# revision 65
# speedup vs baseline: 1.0367x; 1.0367x over previous
"""Linear attention (silu+1 feature map) MultiHeadAttention kernel for 8x TRN2.

Sharding: data-parallel over batch (B=8 -> 1 batch element per NeuronCore).

fp8 DoubleRow design. All four big GEMMs run as fp8e4 DoubleRow matmuls
(contraction 256 per instruction at 0.5 cycles/output-row). Accuracy is
preserved by a mean-split: phi(x) = silu(x)+1 = 1 + d(x), so every fp8
operand that multiplies a full-magnitude tensor is a small delta:

  phase 1 (token tiles of 512, q-tiles and kv-subs interleaved per pair):
    qT[o,t]  = wq8.T @ x8T            (DoubleRow, feature-major out)
    dq[o,t]  = silu(s*qT + s*bq)      (ACT, fp8 out; phi_q = 1 + dq)
    k[t,o]   = x8T.T @ wk8            (DoubleRow, token-major)
    v[t,e]   = x8T.T @ (Wv/4)8 + bv/4 (DoubleRow + DVE bias add, fp8 out;
                                       the /4 keeps kv inside fp8e4 range)
    dk[t,o]  = silu(s*k)              (ACT, fp8 out; phi_k = 1 + dk)
    kv[e,d]_h += v_h.T @ dk_h         (DoubleRow over token sub-tile pairs;
                                       [64,16,64] psum, head h in slot
                                       (h%2)*8 + h//2 so even/odd heads are
                                       contiguous for the 2-op repack)
  corrections (the "1" parts of phi_k):
    kv_full  = kv + S_v[e]/4          S_v = sum_t v_exact[t] computed on host
                                      from the original fp32 x (rank-1 term)
  M stage:
    kv8      = block-diag fp8 repack of kv_full  ([64, 2, 128] per chunk)
    M[d,o]_c = kv8_c .T@ (4*wo2)_pair (DoubleRow, contraction = 2 heads' e)
    m8       = fp8(M)                 (exact scale: (kv/4) @ (4 wo))
  colsum (the "1" part of phi_q): y += colsum(M) as a per-partition bias:
    rsum[e]  = reduce_d kv_ps + 16*S_v  (DVE reduce + add, = rowsum(kv)/4)
    bias[o]  = sum_e rsum[e] (4*wo)[e,o] + bo  (128 N=1 matvecs, bf16)
  phase 2:
    yT[o,t]  = m8.T @ dq + bias       (DoubleRow; bias folded into drains)

Host side: fp8 casts (clip +-240), S_v correction, output upcast bf16->fp32.
"""

import numpy as np
import ml_dtypes

B, T, D = 8, 4096, 1024
H, DH = 16, 64
SCALE = float(DH ** -0.25)
NCORES = 8
P = 128
DC = D // P          # 8 feature chunks
NKP = DC // 2        # 4 k-pairs for DoubleRow
TT = 512             # token tile (phase 1)
NTT = T // TT        # 8 token tiles
NSUB = TT // P       # 4 sub-tiles of 128 tokens

_BF16 = ml_dtypes.bfloat16
_F8 = ml_dtypes.float8_e4m3fn

_CACHE = {}


def _slot(h):
    # kv psum column slot for head h: even heads 0-7, odd heads 8-15
    return (h % 2) * 8 + h // 2


def _split_multi_waits(nc):
    """walrus in this container only encodes ONE sync-wait command per
    instruction. Hoist extra waits onto injected same-engine NOPs placed
    immediately before the instruction (program order on the engine queue
    makes this semantically identical)."""
    import concourse.mybir as mybir

    n_split = 0
    for fn in nc.m.functions:
        for bb in fn.blocks:
            new = []
            changed = False
            for inst in bb.instructions:
                si = inst.sync_info
                waits = list(si.on_wait) if si is not None else []
                if len(waits) > 1:
                    changed = True
                    for j, w in enumerate(waits[:-1]):
                        nop = mybir.InstNoOp(
                            name=f"{inst.name}-sw{j}", ins=[], outs=[]
                        )
                        nop.engine = inst.engine
                        nop.sync_info = mybir.SyncInfo(
                            on_wait=[w], on_update=[]
                        )
                        new.append(nop)
                        n_split += 1
                    inst.sync_info = mybir.SyncInfo(
                        on_wait=[waits[-1]], on_update=list(si.on_update)
                    )
                new.append(inst)
            if changed:
                bb.instructions = new
    return n_split


def _build_program():
    import concourse.bass as bass
    import concourse.mybir as mybir
    from concourse.tile import TileContext

    dt = mybir.dt
    AF = mybir.ActivationFunctionType
    PM = mybir.MatmulPerfMode

    nc = bass.Bass()

    xT_d = nc.dram_tensor("xT", [D, T], dt.float8e4, kind="ExternalInput")
    wq_d = nc.dram_tensor("wq", [D, D], dt.float8e4, kind="ExternalInput")
    wk_d = nc.dram_tensor("wk", [D, D], dt.float8e4, kind="ExternalInput")
    wv_d = nc.dram_tensor("wv", [D, D], dt.float8e4, kind="ExternalInput")
    wo2_d = nc.dram_tensor("wo2", [64, H, D], dt.float8e4, kind="ExternalInput")
    wob2_d = nc.dram_tensor("wob2", [64, H, D], dt.bfloat16, kind="ExternalInput")
    bqs_d = nc.dram_tensor("bqs", [P, DC], dt.float32, kind="ExternalInput")
    bos_d = nc.dram_tensor("bos", [P, DC], dt.float32, kind="ExternalInput")
    bvb_d = nc.dram_tensor("bvb", [P, D], dt.float32, kind="ExternalInput")
    corA_d = nc.dram_tensor("corA", [64, 8, 64], dt.float32, kind="ExternalInput")
    corB_d = nc.dram_tensor("corB", [64, 8, 64], dt.float32, kind="ExternalInput")
    cor64_d = nc.dram_tensor("cor64", [64, H], dt.float32, kind="ExternalInput")
    yT_d = nc.dram_tensor("yT", [D, T], dt.bfloat16, kind="ExternalOutput")

    with TileContext(nc) as tc:
        with (
            tc.tile_pool(name="weights", bufs=1) as wpool,
            tc.tile_pool(name="phiq", bufs=1) as qpool,
            tc.tile_pool(name="msb", bufs=1) as mpool,
            tc.tile_pool(name="xin", bufs=3) as xpool,
            tc.tile_pool(name="kvtiles", bufs=4) as kvpool,
            tc.tile_pool(name="yout", bufs=4) as ypool,
        ):
            # ---- weight / const preload ----
            wq_sb = wpool.tile([P, DC, D], dt.float8e4, tag="wq")
            wk_sb = wpool.tile([P, DC, D], dt.float8e4, tag="wk")
            wv_sb = wpool.tile([P, DC, D], dt.float8e4, tag="wv")
            wo2_sb = wpool.tile([64, H, D], dt.float8e4, tag="wo2")
            wob2_sb = wpool.tile([64, H, D], dt.bfloat16, tag="wob2")
            bq_sb = wpool.tile([P, DC], dt.float32, tag="bq")
            bo_sb = wpool.tile([P, DC], dt.float32, tag="bo")
            bv_sb = wpool.tile([P, D], dt.float32, tag="bv")
            corA_sb = wpool.tile([64, 8, 64], dt.float32, tag="corA")
            corB_sb = wpool.tile([64, 8, 64], dt.float32, tag="corB")
            cor64_sb = wpool.tile([64, H], dt.float32, tag="cor64")
            bias_sb = wpool.tile([P, DC], dt.float32, tag="bias")
            rsum_sb = wpool.tile([64, H], dt.bfloat16, tag="rsum")
            rsraw_sb = wpool.tile([64, H], dt.float32, tag="rsraw")
            qraw_sb = wpool.tile([P, DC, TT], dt.bfloat16, tag="qraw")

            # zdum only feeds warmup matmuls (whose outputs are later
            # overwritten by start=True matmuls); memset it first so the
            # PE can start its warmup right away
            zdum = wpool.tile([1, 640], dt.bfloat16, tag="zdum")
            nc.gpsimd.memset(zdum[:], 0.0)

            wq_r = wq_d.rearrange("(c p) o -> p c o", p=P)
            xT_r = xT_d.rearrange("(c p) t -> p c t", p=P)

            # sync queue: what phase-1 pair 0's q tiles need, in need-order.
            # gpsimd queue: wk/wv (needed ~11us in) + consts + x pairs 1-3.
            xt01 = xpool.tile([P, DC, 2 * TT], dt.float8e4, tag="xt0", bufs=1)
            nc.sync.dma_start(xt01[:, :, 0:TT], xT_r[:, :, 0:TT])
            nc.sync.dma_start(wq_sb[:, :, 0:512], wq_r[:, :, 0:512])
            nc.sync.dma_start(xt01[:, :, TT : 2 * TT], xT_r[:, :, TT : 2 * TT])
            nc.sync.dma_start(wq_sb[:, :, 512:1024], wq_r[:, :, 512:1024])
            nc.sync.dma_start(bv_sb[:], bvb_d[:])
            nc.sync.dma_start(bo_sb[:], bos_d[:])
            nc.sync.dma_start(wo2_sb[:], wo2_d[:])
            nc.sync.dma_start(wob2_sb[:], wob2_d[:])
            nc.gpsimd.dma_start(bq_sb[:], bqs_d[:])
            nc.gpsimd.dma_start(wk_sb[:], wk_d.rearrange("(c p) o -> p c o", p=P))
            nc.gpsimd.dma_start(wv_sb[:], wv_d.rearrange("(c p) o -> p c o", p=P))
            nc.gpsimd.dma_start(corA_sb[:], corA_d[:])
            nc.gpsimd.dma_start(corB_sb[:], corB_d[:])
            nc.gpsimd.dma_start(cor64_sb[:], cor64_d[:])
            xt_pre = [xt01[:, :, 0:TT], xt01[:, :, TT : 2 * TT]]

            phi_q = qpool.tile([P, DC, T], dt.float8e4, tag="phiq")
            m8 = mpool.tile([P, DC, D], dt.float8e4, tag="m8")
            # block-diag fp8 repack of kv: chunk c = [64, 2, 128]; j=0 holds
            # head 2c in cols 0:64, j=1 holds head 2c+1 in cols 64:128, rest 0
            kv8 = mpool.tile([64, DC, 2, P], dt.float8e4, tag="kv8")

            zz = wpool.tile([1, 640], dt.bfloat16, tag="zz")
            nc.vector.memset(zz[:], 0.0)
            nc.gpsimd.memset(kv8[:], 0.0)

            with tc.tile_pool(name="ps_kv", bufs=1, space="PSUM") as pkv_pool:
                # kv accumulator: head h in column slot (h%2)*8 + h//2
                kv_ps = pkv_pool.tile([64, H, 64], dt.float32, tag="kvacc")

                # warmup matmuls filling the startup DMA shadow: keep the PE
                # p-state warm so the first real matmuls run at full clock.
                # They scribble garbage into the kv bank; the first real kv
                # accumulation per head slot uses start=True to overwrite.
                for w in range(44):
                    nc.tensor.matmul(
                        kv_ps[:, 0:2, :], lhsT=zdum[:1, :64], rhs=zdum[:1, 64:192],
                        start=True, stop=True, skip_group_check=True,
                    )

                kv_pend = [None]
                kv_first = [True]

                def _emit_kv(pending, last):
                    v2_p, dk2_p = pending
                    for h in range(H):
                        nc.tensor.matmul(
                            kv_ps[:, _slot(h), :],
                            lhsT=v2_p[:, :, h * 64 : (h + 1) * 64],
                            rhs=dk2_p[:, :, h * 64 : (h + 1) * 64],
                            start=kv_first[0],
                            stop=last and h == H - 1,
                            skip_group_check=True,
                            perf_mode=PM.DoubleRow,
                        )
                    kv_first[0] = False

                with tc.tile_pool(name="ps_big", bufs=3, space="PSUM") as pbig:

                    def _q_tile(pair, xts, oc, defer_half=False):
                        pq = pbig.tile([P, 2 * TT], dt.float32, tag="pbig")
                        for half in range(2):
                            for kp in range(NKP):
                                nc.tensor.matmul(
                                    pq[:, half * TT : (half + 1) * TT],
                                    lhsT=wq_sb[:, 2 * kp : 2 * kp + 2, oc * P : (oc + 1) * P],
                                    rhs=xts[half][:, 2 * kp : 2 * kp + 2, :],
                                    start=(kp == 0),
                                    stop=(kp == NKP - 1),
                                    perf_mode=PM.DoubleRow,
                                )
                        if defer_half:
                            # ACT is saturated in pair 0's q stretch: silu
                            # the first token half now, park the second as
                            # raw bf16 via the idle DVE and silu it during
                            # the kv section
                            nc.scalar.activation(
                                phi_q[:, oc, pair * 2 * TT : pair * 2 * TT + TT],
                                pq[:, 0:TT], AF.Silu,
                                bias=bq_sb[:, oc : oc + 1], scale=SCALE,
                            )
                            nc.vector.tensor_copy(
                                out=qraw_sb[:, oc, :], in_=pq[:, TT : 2 * TT]
                            )
                        else:
                            nc.scalar.activation(
                                phi_q[:, oc, pair * 2 * TT : (pair + 1) * 2 * TT],
                                pq[:], AF.Silu,
                                bias=bq_sb[:, oc : oc + 1], scale=SCALE,
                            )

                    def _q_deferred(pair, oc):
                        nc.scalar.activation(
                            phi_q[:, oc, pair * 2 * TT + TT : (pair + 1) * 2 * TT],
                            qraw_sb[:, oc, :], AF.Silu,
                            bias=bq_sb[:, oc : oc + 1], scale=SCALE,
                        )

                    kv_cur = [None, None]

                    def _kv_sub(pair, xts, g):
                        half, sub = divmod(g, NSUB)
                        j = g % 2
                        xs = xts[half][:, :, sub * P : (sub + 1) * P]
                        pk = pbig.tile([P, D], dt.float32, tag="pbig")
                        pv = pbig.tile([P, D], dt.float32, tag="pbig")
                        for n in range(2):
                            for kp in range(NKP):
                                nc.tensor.matmul(
                                    pk[:, n * 512 : (n + 1) * 512],
                                    lhsT=xs[:, 2 * kp : 2 * kp + 2, :],
                                    rhs=wk_sb[:, 2 * kp : 2 * kp + 2, n * 512 : (n + 1) * 512],
                                    start=(kp == 0),
                                    stop=(kp == NKP - 1),
                                    perf_mode=PM.DoubleRow,
                                )
                        for n in range(2):
                            for kp in range(NKP):
                                nc.tensor.matmul(
                                    pv[:, n * 512 : (n + 1) * 512],
                                    lhsT=xs[:, 2 * kp : 2 * kp + 2, :],
                                    rhs=wv_sb[:, 2 * kp : 2 * kp + 2, n * 512 : (n + 1) * 512],
                                    start=(kp == 0),
                                    stop=(kp == NKP - 1),
                                    perf_mode=PM.DoubleRow,
                                )
                        if j == 0:
                            kv_cur[0] = kvpool.tile(
                                [P, 2, D], dt.float8e4, tag="v2", name="v2"
                            )
                            kv_cur[1] = kvpool.tile(
                                [P, 2, D], dt.float8e4, tag="dk2", name="dk2"
                            )
                        v2, dk2 = kv_cur
                        nc.scalar.activation(
                            dk2[:, j, :], pk[:], AF.Silu, scale=SCALE
                        )
                        nc.vector.tensor_add(v2[:, j, :], pv[:], bv_sb[:])
                        if j == 1:
                            if kv_pend[0] is not None:
                                _emit_kv(kv_pend[0], False)
                            kv_pend[0] = (v2, dk2)

                    def _hook_flush():
                        _emit_kv(kv_pend[0], True)
                        kv_pend[0] = None

                    def _hook_repack():
                        # kv_ps holds kv/4 (v pre-scaled); straight add+cast.
                        # gpsimd cannot read PSUM, so everything is ACT/DVE;
                        # chunks 0-1 go first since the M stage gates on them
                        nc.vector.tensor_add(
                            kv8[:, 0:2, 0, 0:64], kv_ps[:, 0:2, :],
                            corA_sb[:, 0:2, :],
                        )
                        nc.vector.tensor_add(
                            kv8[:, 0:2, 1, 64:128], kv_ps[:, 8:10, :],
                            corB_sb[:, 0:2, :],
                        )
                        nc.vector.tensor_add(
                            kv8[:, 2:8, 0, 0:64], kv_ps[:, 2:8, :],
                            corA_sb[:, 2:8, :],
                        )
                        nc.vector.tensor_add(
                            kv8[:, 2:8, 1, 64:128], kv_ps[:, 10:16, :],
                            corB_sb[:, 2:8, :],
                        )
                        nc.vector.tensor_reduce(
                            rsraw_sb[:], kv_ps[:],
                            axis=mybir.AxisListType.X,
                            op=mybir.AluOpType.add,
                        )
                        nc.vector.tensor_add(
                            rsum_sb[:], rsraw_sb[:], cor64_sb[:]
                        )

                    def _m_stage(cs):
                        # half-drains on ACT+DVE in parallel free the psum
                        # slot ~2x sooner than a single 1024-wide copy
                        for c in cs:
                            pm = pbig.tile([P, D], dt.float32, tag="pbig")
                            for n in range(2):
                                nc.tensor.matmul(
                                    pm[:, n * 512 : (n + 1) * 512],
                                    lhsT=kv8[:, c, :, :],
                                    rhs=wo2_sb[:, 2 * c : 2 * c + 2, n * 512 : (n + 1) * 512],
                                    start=True,
                                    stop=True,
                                    perf_mode=PM.DoubleRow,
                                )
                            nc.scalar.copy(out=m8[:, c, 0:512], in_=pm[:, 0:512])
                            nc.vector.tensor_copy(
                                out=m8[:, c, 512:1024], in_=pm[:, 512:1024]
                            )

                    def _hook_bias():
                        bt = pbig.tile([P, D], dt.float32, tag="pbig")
                        nc.tensor.matmul(
                            bt[:, 0:8], lhsT=zz[:1, :P], rhs=zz[:1, P : P + 8],
                            start=True, stop=True, skip_group_check=True,
                        )
                        for oc in range(DC):
                            for jj in range(H):
                                nc.tensor.matmul(
                                    bt[:, oc : oc + 1],
                                    lhsT=wob2_sb[:, jj, oc * P : (oc + 1) * P],
                                    rhs=rsum_sb[:, jj : jj + 1],
                                    start=False,
                                    stop=(oc == DC - 1 and jj == H - 1),
                                    skip_group_check=True,
                                )
                        nc.vector.tensor_add(bias_sb[:], bt[:, 0:8], bo_sb[:])

                    # ---- phase-1 pair schedules ----
                    for pair in range(NTT // 2):
                        if pair == 0:
                            xts = xt_pre
                        else:
                            xts = []
                            for half in range(2):
                                tt = pair * 2 + half
                                xt = xpool.tile([P, DC, TT], dt.float8e4, tag="xt")
                                nc.gpsimd.dma_start(
                                    xt[:], xT_r[:, :, tt * TT : (tt + 1) * TT]
                                )
                                xts.append(xt)

                        if pair == 0:
                            # startup: q tiles first (kv weights still
                            # loading), then the kv subs with the deferred
                            # second-half silus woven in
                            for oc in range(DC):
                                _q_tile(pair, xts, oc, defer_half=True)
                            for g in range(2 * NSUB):
                                _kv_sub(pair, xts, g)
                                _q_deferred(pair, g)
                        elif pair < NTT // 2 - 1:
                            # interleave 1:1 to balance the ACT engine
                            for u in range(DC):
                                _q_tile(pair, xts, u)
                                _kv_sub(pair, xts, u)
                        else:
                            # last pair: all kv subs first so the kv flush,
                            # repack, M stage and colsum hide under the q
                            # tiles that run right before phase 2.
                            for g in range(2 * NSUB):
                                _kv_sub(pair, xts, g)
                            _q_tile(pair, xts, 0)
                            _hook_flush()
                            _q_tile(pair, xts, 1)
                            _hook_repack()
                            _q_tile(pair, xts, 2)
                            _m_stage((0, 1))
                            _q_tile(pair, xts, 3)
                            _m_stage((2, 3))
                            _q_tile(pair, xts, 4)
                            _m_stage((4, 5))
                            _q_tile(pair, xts, 5)
                            _m_stage((6, 7))
                            _hook_bias()
                            _q_tile(pair, xts, 6)
                            _q_tile(pair, xts, 7)

                    # ---- phase 2: yT = m8.T @ dq + bias ----
                    # kp rotated per tile so the first tiles only need the
                    # early m8 chunks. Two qb tiles share one [P, 2048]
                    # output buffer -> half the DMAs at double the size;
                    # queues alternate sync/gpsimd. The last tile drains in
                    # 512-col pieces with its own small DMAs so the kernel
                    # tail stays short.
                    nt2 = 0
                    for oc in range(DC):
                        for qp in range(2):
                            # last two buffer-pairs use small per-half
                            # transfers so the DMA pipe drains quickly at
                            # the kernel end
                            idx = oc * 2 + qp
                            last_pair = idx >= 14
                            ysb = ypool.tile([P, 4 * TT], dt.bfloat16, tag="ysb")
                            for qh in range(2):
                                qb = qp * 2 + qh
                                py = pbig.tile([P, 2 * TT], dt.float32, tag="pbig")
                                for half in range(2):
                                    for i in range(NKP):
                                        kp = (nt2 + i) % NKP
                                        nc.tensor.matmul(
                                            py[:, half * TT : (half + 1) * TT],
                                            lhsT=m8[:, 2 * kp : 2 * kp + 2, oc * P : (oc + 1) * P],
                                            rhs=phi_q[:, 2 * kp : 2 * kp + 2,
                                                      qb * 1024 + half * TT : qb * 1024 + (half + 1) * TT],
                                            start=(i == 0),
                                            stop=(i == NKP - 1),
                                            perf_mode=PM.DoubleRow,
                                        )
                                nt2 += 1
                                # full-tile drains alternating ACT/DVE keep
                                # both engines under ~75% so neither builds
                                # a backlog; the final tile splits for a
                                # short tail.
                                o0 = qh * 2 * TT
                                if last_pair:
                                    # final pairs: fine-grained half drains,
                                    # transfers fanned over the queues
                                    nc.scalar.activation(
                                        ysb[:, o0 : o0 + TT], py[:, 0:TT],
                                        AF.Identity,
                                        bias=bias_sb[:, oc : oc + 1], scale=1.0,
                                    )
                                    nc.vector.tensor_scalar_add(
                                        ysb[:, o0 + TT : o0 + 2 * TT],
                                        py[:, TT : 2 * TT],
                                        bias_sb[:, oc : oc + 1],
                                    )
                                    q0 = nc.sync if qh == 0 else nc.scalar
                                    q1 = nc.gpsimd if qh == 0 else nc.sync
                                    q0.dma_start(
                                        yT_d[oc * P : (oc + 1) * P,
                                             qb * 1024 : qb * 1024 + TT],
                                        ysb[:, o0 : o0 + TT],
                                    )
                                    q1.dma_start(
                                        yT_d[oc * P : (oc + 1) * P,
                                             qb * 1024 + TT : (qb + 1) * 1024],
                                        ysb[:, o0 + TT : o0 + 2 * TT],
                                    )
                                elif qh == 0:
                                    nc.scalar.activation(
                                        ysb[:, 0 : 2 * TT], py[:],
                                        AF.Identity,
                                        bias=bias_sb[:, oc : oc + 1], scale=1.0,
                                    )
                                else:
                                    nc.vector.tensor_scalar_add(
                                        ysb[:, 2 * TT : 4 * TT], py[:],
                                        bias_sb[:, oc : oc + 1],
                                    )
                            if last_pair:
                                pass
                            elif (oc * 2 + qp) % 2 == 0:
                                nc.sync.dma_start(
                                    yT_d[oc * P : (oc + 1) * P,
                                         qp * 2048 : (qp + 1) * 2048],
                                    ysb[:],
                                )
                            else:
                                nc.gpsimd.dma_start(
                                    yT_d[oc * P : (oc + 1) * P,
                                         qp * 2048 : (qp + 1) * 2048],
                                    ysb[:],
                                )
    _split_multi_waits(nc)
    return nc


def _get_program():
    key = "nc"
    if key not in _CACHE:
        _CACHE[key] = _build_program()
    return _CACHE[key]


def _q8(a):
    return np.clip(np.asarray(a, np.float32), -240.0, 240.0).astype(_F8)


def _head_order():
    # slot j -> head: even heads first (0,2,..,14), then odd (1,3,..,15)
    return [2 * j for j in range(8)] + [2 * j + 1 for j in range(8)]


def _prep_shared(Wq, bq, Wk, Wv, bv, Wo, bo):
    woT = np.ascontiguousarray(Wo.T)  # [e, o]
    wo_heads = woT.reshape(H, 64, D)
    shared = {
        "wq": _q8(np.ascontiguousarray(Wq.T)),
        "wk": _q8(np.ascontiguousarray(Wk.T)),
        "wv": _q8(0.25 * np.ascontiguousarray(Wv.T)),
        "wo2": _q8(4.0 * wo_heads.transpose(1, 0, 2)),
        "wob2": np.ascontiguousarray(
            4.0 * wo_heads[_head_order()].transpose(1, 0, 2)
        ).astype(_BF16),
        "bqs": np.ascontiguousarray(
            (SCALE * bq).astype(np.float32).reshape(DC, P).T
        ),
        "bos": np.ascontiguousarray(bo.astype(np.float32).reshape(DC, P).T),
        "bvb": np.ascontiguousarray(
            np.broadcast_to(0.25 * bv.astype(np.float32), (P, D))
        ),
    }
    return shared


def _run(in_maps, **kw):
    from concourse.bass_utils import run_bass_kernel_spmd

    nc = _get_program()
    return run_bass_kernel_spmd(nc, in_maps, list(range(NCORES)), **kw)


def kernel(x, Wq, bq, Wk, Wv, bv, Wo, bo):
    x = np.asarray(x, dtype=np.float32)
    assert x.shape == (B, T, D), x.shape
    Wv = np.asarray(Wv, np.float32)
    bv = np.asarray(bv, np.float32)
    shared = _prep_shared(
        np.asarray(Wq, np.float32), np.asarray(bq, np.float32),
        np.asarray(Wk, np.float32), Wv, bv,
        np.asarray(Wo, np.float32), np.asarray(bo, np.float32),
    )
    ho = _head_order()
    in_maps = []
    for b in range(B):
        m = dict(shared)
        m["xT"] = _q8(x[b].T)
        # exact rank-1 kv correction: S_v[e] = sum_t v_exact[t, e]; kv is
        # accumulated at 1/4 scale on-device
        sv = (x[b].astype(np.float64).sum(0) @ Wv.T.astype(np.float64)
              + T * bv.astype(np.float64)).astype(np.float32)
        svh = 0.25 * sv.reshape(H, 64)                    # [head, e] / 4
        m["corA"] = np.ascontiguousarray(
            np.broadcast_to(svh[0::2].transpose(1, 0)[:, :, None], (64, 8, 64))
        )
        m["corB"] = np.ascontiguousarray(
            np.broadcast_to(svh[1::2].transpose(1, 0)[:, :, None], (64, 8, 64))
        )
        m["cor64"] = np.ascontiguousarray(64.0 * svh[ho].transpose(1, 0))
        in_maps.append(m)

    res = _run(in_maps)
    out = np.empty((B, T, D), np.float32)
    for b in range(B):
        out[b] = res.results[b]["yT"].astype(np.float32).T
    return out
